# revision 71
# baseline (speedup 1.0000x reference)
"""Multi-head self-attention with positional bias, sharded over 8 NeuronCores.

Sharding: head-parallel. Core h computes head h for all batches:
  q/k/v projections with the head's weight slices, scores + softmax with the
  head's pos_bias slice, and the partial output  o_h @ Wout[h*64:(h+1)*64, :].
The full output is the sum of the 8 partials (row-parallel Wout).

Device kernel (per core), all matmuls bf16 (fp32 PSUM accumulation):
  - query supplied pre-transposed (qT [D, B*N] bf16): projection contraction
    on SBUF partitions.
  - q/k projections PACKED: stationary [Wq*scale | Wk] -> one [128, tokens]
    PSUM tile per chunk, evacuated into qkT_sb[pair] [128, 2, n] (k at index
    0, q at 1; batch lb on partition half lb) so the score matmul's lhsT and
    rhs share a base partition.
  - v untransposed ([tokens, dh]) + 65th ones-column -> free softmax denom.
  - scores TRANSPOSED: ST[j, i] = k_j . q_i, so exp's output P~[j, i] is
    directly the attention*V layout.
  - bias folded as exp(S + B) = exp(S) * exp(B): host supplies
    EB = exp(pos_bias^T) bf16; DVE multiplies it in (bf16 2x mode).
  - exp skips max-subtraction (scores ~N(0,2), safe in f32/bf16).
  - Software pipelining inside each (iw, pair, lb) loop: slot jt runs
    qk/exp/mult and the AV matmul for i-half 0 of tile jt, the AV matmul for
    i-half 1 of tile jt-1 (a second PSUM accumulation group lagging one
    slot), and one Wout-projection tile of the PREVIOUS loop. Projections of
    batches 2-3 are interleaved into loops L1/L2. PSUM rings: st[128,1024]x2,
    ot[65,512]x2, po[128,512]x2 = 16KB exactly.
  - normalization deferred to the po evacuation (per-partition scalar mul),
    denominators bounced through DRAM to reach token-partition layout.
"""

import numpy as np
from contextlib import ExitStack

import concourse.bass as bass
import concourse.bacc as bacc
import concourse.mybir as mybir
import concourse.tile as tile
from concourse.bass_utils import run_bass_kernel_spmd

HEADS = 8
DH = 64
B, N, D = 4, 2048, 512
SCALE = DH ** -0.5
N_CORES = 8
IW = 1024           # i-window (query tokens per score tile)
VW = DH + 1         # v block width (+1 ones column for the denominator)

F32 = mybir.dt.float32
BF16 = mybir.dt.bfloat16
EXP = mybir.ActivationFunctionType.Exp
COPY = mybir.ActivationFunctionType.Copy


def build_nc(b=B, n=N, d=D, packed=False, n_cores=1):
    assert b % 2 == 0 and n % IW == 0 and d % 256 == 0
    CC = d // 128       # contraction chunks for the projections
    CH = CC // 2        # chunks per qt half-tile
    NJ = n // 128       # key tiles (j)
    NIW = n // IW       # i-windows per batch
    NPAIR = b // 2
    NTW = IW // 128     # token tiles per i-window
    T = b * n

    nc = bacc.Bacc("TRN2", target_bir_lowering=False, debug=False,
                   num_devices=n_cores)
    qT = nc.declare_dram_parameter("qT", [d, T], BF16, isOutput=False)
    ebT = nc.declare_dram_parameter("ebT", [n, n], BF16, isOutput=False)
    wqk = nc.declare_dram_parameter("wqk", [d, 128], BF16, isOutput=False)
    wv = nc.declare_dram_parameter("wv", [d, DH], BF16, isOutput=False)
    wout = nc.declare_dram_parameter("wout", [DH, d], BF16, isOutput=False)
    out = nc.declare_dram_parameter("out", [T, d], BF16, isOutput=True)

    with ExitStack() as ctx:
        tc = ctx.enter_context(tile.TileContext(nc))

        const = ctx.enter_context(tc.tile_pool(name="const", bufs=1))
        qk_pool = ctx.enter_context(tc.tile_pool(name="qkT", bufs=1))
        v_pool = ctx.enter_context(tc.tile_pool(name="v", bufs=1))
        ot_sb_pool = ctx.enter_context(tc.tile_pool(name="otsb", bufs=1))
        qt_pool = ctx.enter_context(tc.tile_pool(name="qt", bufs=6))
        eb_pool = ctx.enter_context(tc.tile_pool(name="eb", bufs=8))
        praw_pool = ctx.enter_context(tc.tile_pool(name="praw", bufs=6))
        p_pool = ctx.enter_context(tc.tile_pool(name="pexp", bufs=10))
        out_pool = ctx.enter_context(tc.tile_pool(name="osb", bufs=4))
        st_pool = ctx.enter_context(tc.tile_pool(name="st", bufs=2, space="PSUM"))
        ot_pool = ctx.enter_context(tc.tile_pool(name="ot", bufs=2, space="PSUM"))
        po_pool = ctx.enter_context(tc.tile_pool(name="po", bufs=2, space="PSUM"))

        zbias = const.tile([128, 1], F32, tag="zbias")
        nc.vector.memset(zbias, 0.0)
        ones16 = const.tile([128, 16], F32, tag="ones16")
        nc.vector.memset(ones16, 1.0)
        ones_bf = const.tile([128, 1], BF16, tag="ones_bf")
        nc.vector.memset(ones_bf, 1.0)

        wqk_sb = const.tile([128, CC * 128], BF16, tag="wqk")
        nc.sync.dma_start(
            out=wqk_sb.rearrange("p (c e) -> p c e", c=CC),
            in_=wqk[:, :].rearrange("(c p) e -> p c e", p=128))
        wv_sb = const.tile([128, CC * DH], BF16, tag="wv")
        nc.sync.dma_start(
            out=wv_sb.rearrange("p (c e) -> p c e", c=CC),
            in_=wv[:, :].rearrange("(c p) e -> p c e", p=128))
        wout_sb = const.tile([64, d], BF16, tag="wout")
        nc.sync.dma_start(out=wout_sb, in_=wout[:, :])

        # qkT_sb[pair][rows(lb), 0:n] = kT, [rows(lb), n:2n] = qT
        qkT_sb = [qk_pool.tile([128, 2 * n], BF16, tag=f"qkT{p}", name=f"qkT{p}")
                  for p in range(NPAIR)]
        v_sb = [v_pool.tile([128, NJ * VW], BF16, tag=f"v{bb}", name=f"v{bb}")
                for bb in range(b)]
        for bb in range(b):
            ones_cols = v_sb[bb].rearrange("p (t w) -> p t w", w=VW)[:, :, DH:VW]
            nc.gpsimd.tensor_copy(
                ones_cols, ones16[:, 0:NJ].rearrange("p (t o) -> p t o", o=1))
        # attention output, rows 0:64 = oT (dh), row 64 = denominator
        ot_sb = [ot_sb_pool.tile([VW, n], BF16, tag=f"ot{bb}", name=f"ot{bb}")
                 for bb in range(b)]

        recip_sb = [const.tile([128, NJ], F32, tag=f"rs{bb}", name=f"rs{bb}")
                    for bb in range(b)]

        # PE p-state warmup: a dependency-free matmul chain so the PE hits
        # full clock before the first projection (results are discarded).
        warm = const.tile([128, 512], BF16, tag="warm")
        nc.vector.memset(warm, 0.0)
        for r in range(8):
            wt = st_pool.tile([128, 512], F32, tag="st", name="warm")
            nc.tensor.matmul(wt, lhsT=warm[:, 0:128], rhs=warm,
                             start=True, stop=True)

        # ---------------- projections ----------------
        qt_tiles = {}

        def qt_fetch(bb, g=None):
            if g is None:
                for gg in range(n // 1024):
                    qt_fetch(bb, gg)
                return
            t = qt_pool.tile([128, CC * 1024], BF16, tag="qt", name="qt")
            nc.sync.dma_start(
                out=t.rearrange("p (c t) -> p c t", c=CC),
                in_=qT[:, bb * n + g * 1024: bb * n + (g + 1) * 1024]
                .rearrange("(c p) t -> p c t", p=128))
            qt_tiles[(bb, g)] = t

        def proj_group(bb, g, evac_eng):
            """Project one 1024-token group of batch bb (q/k packed + v)."""
            pair, lb = bb // 2, bb % 2
            rows = slice(64 * lb, 64 * lb + 64)
            qt_t = qt_tiles[(bb, g)]
            e1, e2 = evac_eng
            for half in range(2):
                g0 = g * 1024 + half * 512
                l0 = half * 512
                ps = po_pool.tile([128, 512], F32, tag="po", name="pqk")
                for c in range(CC):
                    nc.tensor.matmul(
                        ps, lhsT=wqk_sb[:, c * 128:(c + 1) * 128],
                        rhs=qt_t[:, c * 1024 + l0: c * 1024 + l0 + 512],
                        start=(c == 0), stop=(c == CC - 1))
                if e1 is nc.scalar:
                    nc.scalar.copy(qkT_sb[pair][rows, n + g0: n + g0 + 512],
                                   ps[0:64, :])
                else:
                    nc.vector.tensor_copy(
                        qkT_sb[pair][rows, n + g0: n + g0 + 512], ps[0:64, :])
                if e2 is nc.scalar:
                    nc.scalar.copy(qkT_sb[pair][rows, g0: g0 + 512],
                                   ps[64:128, :])
                else:
                    nc.vector.tensor_copy(
                        qkT_sb[pair][rows, g0: g0 + 512], ps[64:128, :])
            psv = po_pool.tile([128, 8 * DH], F32, tag="po", name="psv")
            for tt in range(8):
                for c in range(CC):
                    nc.tensor.matmul(
                        psv[:, tt * DH:(tt + 1) * DH],
                        lhsT=qt_t[:, c * 1024 + tt * 128:
                                  c * 1024 + (tt + 1) * 128],
                        rhs=wv_sb[:, c * DH:(c + 1) * DH],
                        start=(c == 0), stop=(c == CC - 1))
            vdst = v_sb[bb].rearrange("p (t w) -> p t w", w=VW)[
                :, g * 8:(g + 1) * 8, 0:DH]
            if e1 is nc.scalar:
                nc.scalar.copy(vdst, psv.rearrange("p (t w) -> p t w", w=DH))
            else:
                nc.vector.tensor_copy(vdst, psv.rearrange("p (t w) -> p t w", w=DH))

        eb_t = {}

        def eb_fetch_singles(iw, jts):
            for jt in jts:
                t = eb_pool.tile([128, IW], BF16, tag="ebs", name="ebs",
                                 bufs=NJ)
                nc.sync.dma_start(
                    out=t, in_=ebT[jt * 128:(jt + 1) * 128,
                                   iw * IW:(iw + 1) * IW])
                eb_t[(iw, jt)] = t

        def eb_fetch_pack(iw, jg):
            t = eb_pool.tile([128, 4 * IW], BF16, tag="eb", name="eb", bufs=4)
            nc.sync.dma_start(
                out=t.rearrange("p (a i) -> p a i", a=4),
                in_=ebT[jg * 512:(jg + 1) * 512,
                        iw * IW:(iw + 1) * IW]
                .rearrange("(a p) i -> p a i", p=128))
            for u in range(4):
                eb_t[(iw, jg * 4 + u)] = t[:, u * IW:(u + 1) * IW]

        # Head: fetch/project pair0 and prefetch eb(iw0) with interleaved DMAs
        # so loop L0 can start ~10us in; batches 2-3 project inside L1/L2.
        NGH = n // 1024
        qt_fetch(0, 0)
        eb_fetch_singles(0, range(0, NJ // 4))
        for g in range(1, NGH):
            qt_fetch(0, g)
        eb_fetch_singles(0, range(NJ // 4, NJ // 2))
        qt_fetch(1, 0)
        eb_fetch_singles(0, range(NJ // 2, 3 * NJ // 4))
        for g in range(1, NGH):
            qt_fetch(1, g)
        eb_fetch_singles(0, range(3 * NJ // 4, NJ))
        for bb in range(2, b):
            qt_fetch(bb)
        for g in range(n // 1024):
            proj_group(0, g, (nc.vector, nc.vector))
        for bb in range(4, b):  # b > 4: project remaining batches upfront
            for g in range(n // 1024):
                proj_group(bb, g, (nc.vector, nc.scalar))

        # -------- pipelined attention + output projection loops --------
        NL = NIW * NPAIR * 2
        loops = [(iw, pair, lb) for iw in range(NIW) for pair in range(NPAIR)
                 for lb in range(2)]
        NG = n // 1024  # proj groups per batch

        def extra_work(i, jt):
            """Work stolen from later phases, interleaved into slot (i, jt)."""
            if i in (0, 1, 2) and i + 1 < b:
                bbp = i + 1  # project batch 1 in L0, 2 in L1, 3 in L2
                engs = (nc.vector, nc.vector) if i == 0 else (nc.vector, nc.scalar)
                if i == 0:
                    for g in range(NG):
                        if jt == NJ - 1 - 4 * (NG - 1 - g):
                            proj_group(bbp, g, engs)
                elif jt % (NJ // NG) == NJ // NG - 1 and jt * NG // NJ < NG:
                    proj_group(bbp, jt * NG // NJ, engs)
            if iw_of[i] + 1 < NIW and i < NJ // 4 and jt == NJ - 3:
                eb_fetch_pack(iw_of[i] + 1, i)

        iw_of = [l[0] for l in loops]
        osb_cur = {}

        def po_step(i, tg, ring=None):
            """One Wout-projection tile of loop Li (tg-th token tile)."""
            iw, pair, lb = loops[i]
            bb = 2 * pair + lb
            gtg = iw * NTW + tg
            pool_, tag_ = (st_pool, "st") if ring == "st" else (po_pool, "po")
            po = pool_.tile([128, d], F32, tag=tag_, name="po")
            nc.tensor.matmul(
                po, lhsT=ot_sb[bb][0:64, gtg * 128:(gtg + 1) * 128],
                rhs=wout_sb, start=True, stop=True)
            if i not in osb_cur:
                osb_cur[i] = (out_pool.tile([128, 4 * d], BF16, tag="osb",
                                            name="osb"), 0)
            osb, off = osb_cur[i]
            if i >= NL - 2 and tg % 2 == 1:
                nc.scalar.activation(osb[:, off * d:(off + 1) * d], po, COPY,
                                     scale=recip_sb[bb][:, gtg:gtg + 1])
            else:
                nc.vector.tensor_scalar_mul(osb[:, off * d:(off + 1) * d], po,
                                            recip_sb[bb][:, gtg:gtg + 1])
            if off == 3:
                row0 = bb * n + (gtg - 3) * 128
                nc.sync.dma_start(
                    out=out[row0: row0 + 512, :]
                    .rearrange("(g p) f -> p g f", p=128),
                    in_=osb.rearrange("p (g f) -> p g f", g=4))
                del osb_cur[i]
            else:
                osb_cur[i] = (osb, off + 1)

        # po slot schedule: spread the NTW projection tiles of the previous
        # loop over slots [POS, NJ)
        POS = max(2, min(8, NJ - NTW))
        po_slots = {}
        for tg in range(NTW):
            po_slots.setdefault(POS + tg * (NJ - POS) // NTW, []).append(tg)

        def den_recip(bb, dwin, twin):
            """Transpose the denominator row into token-partition layout with
            tiny PE matmuls (den_col^T @ 1) and take the reciprocal."""
            w = twin.stop - twin.start
            dn = po_pool.tile([128, 512], F32, tag="po", name="dn")
            for k in range(w):
                nc.tensor.matmul(
                    dn[:, k:k + 1],
                    lhsT=ot_sb[bb][64:65, dwin.start + k * 128:
                                   dwin.start + (k + 1) * 128],
                    rhs=ones_bf[64:65, 0:1], start=True, stop=True)
            nc.vector.reciprocal(recip_sb[bb][:, twin], dn[:, 0:w])

        def end_work(i, part):
            """Deferred loop-end work of Li: part 0 = evac ot0; part 1 =
            evac ot1 + denominator bounce + reciprocal."""
            iw, pair, lb = loops[i]
            bb = 2 * pair + lb
            if part == 0:
                nc.vector.tensor_copy(
                    ot_sb[bb][:, iw * IW: iw * IW + 512],
                    ot_tiles.pop((i, 0))[0:VW, :])
                if i != NL - 1:
                    return
            if part == 0 and i == NL - 1:
                den_recip(bb, slice(iw * IW, iw * IW + 512),
                          slice(iw * NTW, iw * NTW + NTW // 2))
                return
            nc.vector.tensor_copy(
                ot_sb[bb][:, iw * IW + 512: iw * IW + 1024],
                ot_tiles.pop((i, 1))[0:VW, :])
            if i == NL - 1:
                den_recip(bb, slice(iw * IW + 512, (iw + 1) * IW),
                          slice(iw * NTW + NTW // 2, (iw + 1) * NTW))
            else:
                den_recip(bb, slice(iw * IW, (iw + 1) * IW),
                          slice(iw * NTW, (iw + 1) * NTW))

        ot_tiles = {}
        for i in range(NL):
            iw, pair, lb = loops[i]
            bb = 2 * pair + lb
            rows = slice(64 * lb, 64 * lb + 64)
            lag0 = 4 if i == 0 else 2
            lag1 = lag0 + 2
            hist = {}
            for jt in range(NJ):
                st = st_pool.tile([128, IW], F32, tag="st")
                for ih in range(IW // 512):
                    nc.tensor.matmul(
                        st[:, ih * 512:(ih + 1) * 512],
                        lhsT=qkT_sb[pair][rows, jt * 128:(jt + 1) * 128],
                        rhs=qkT_sb[pair][rows, n + iw * IW + ih * 512:
                                         n + iw * IW + (ih + 1) * 512],
                        start=True, stop=True)
                praw = praw_pool.tile([128, IW], BF16, tag="praw")
                nc.scalar.activation(praw, st, EXP, bias=zbias)
                pexp = p_pool.tile([128, IW], BF16, tag="pexp")
                nc.vector.tensor_mul(pexp, praw, eb_t[(iw, jt)])
                hist[jt] = pexp
                if jt == 1 and i > 0:
                    end_work(i - 1, 0)  # frees the ot slot reused by ot0(Li)
                if jt == 2 and i > 0:
                    end_work(i - 1, 1)  # frees the slot reused by ot1(Li)
                av0 = jt - lag0
                if av0 >= 0:
                    if av0 == 0:
                        ot_tiles[(i, 0)] = ot_pool.tile([VW, 512], F32,
                                                        tag="ot", name="ot0")
                    nc.tensor.matmul(
                        ot_tiles[(i, 0)],
                        lhsT=v_sb[bb][:, av0 * VW: av0 * VW + VW],
                        rhs=hist[av0][:, 0:512],
                        start=(av0 == 0), stop=(av0 == NJ - 1))
                av1 = jt - lag1
                if av1 >= 0:
                    if av1 == 0:
                        ot_tiles[(i, 1)] = ot_pool.tile([VW, 512], F32,
                                                        tag="ot", name="ot1")
                    nc.tensor.matmul(
                        ot_tiles[(i, 1)],
                        lhsT=v_sb[bb][:, av1 * VW: av1 * VW + VW],
                        rhs=hist[av1][:, 512:1024],
                        start=(av1 == 0), stop=(av1 == NJ - 1))
                    del hist[av1]
                if i > 1:
                    if i == 3 and NL > 4 and jt in (1, 2, 3):
                        po_step(0, jt + 4)  # po(L0) tg 5..7 deferred from L2
                    for tg in po_slots.get(jt, []):
                        if i == 2 and NL > 4 and tg >= 5:
                            continue  # L2 is overloaded; deferred to L3
                        po_step(i - 2, tg)
                if i == NL - 1 and jt >= NJ // 2:
                    tgx = (jt - NJ // 2) * NTW // (NJ - NJ // 2)
                    tgy = (jt + 1 - NJ // 2) * NTW // (NJ - NJ // 2)
                    for tg in range(tgx, tgy):
                        po_step(i - 1, tg)
                extra_work(i, jt)
            # flush the lagging AV groups
            for jtx in range(NJ - lag0, NJ):
                nc.tensor.matmul(
                    ot_tiles[(i, 0)],
                    lhsT=v_sb[bb][:, jtx * VW: jtx * VW + VW],
                    rhs=hist[jtx][:, 0:512],
                    start=(jtx == 0), stop=(jtx == NJ - 1))
            for jtx in range(NJ - lag1, NJ):
                nc.tensor.matmul(
                    ot_tiles[(i, 1)],
                    lhsT=v_sb[bb][:, jtx * VW: jtx * VW + VW],
                    rhs=hist[jtx][:, 512:1024],
                    start=(jtx == 0), stop=(jtx == NJ - 1))

        # ---------------- drain: finish the last two loops ----------------
        end_work(NL - 1, 0)
        for tg in range(NTW // 2):
            po_step(NL - 1, tg, ring="st" if tg % 2 else None)
        end_work(NL - 1, 1)
        for tg in range(NTW // 2, NTW):
            po_step(NL - 1, tg, ring="st" if tg % 2 else None)
    nc.compile()
    return nc


def make_in_maps(query, pos_bias, Wq, Wk, Wv, Wout, n_cores=N_CORES):
    """Host-side sharding/layout prep. Head h -> core h."""
    import ml_dtypes
    bf16 = ml_dtypes.bfloat16

    query = np.asarray(query, dtype=np.float32)
    pos_bias = np.asarray(pos_bias, dtype=np.float32)
    Wq = np.asarray(Wq, dtype=np.float32)
    Wk = np.asarray(Wk, dtype=np.float32)
    Wv = np.asarray(Wv, dtype=np.float32)
    Wout = np.asarray(Wout, dtype=np.float32)

    b, n, d = query.shape
    qT = np.ascontiguousarray(query.reshape(b * n, d).T).astype(bf16)
    wq_s = Wq * np.float32(SCALE)
    in_maps = []
    for h in range(n_cores):
        sl = slice(h * DH, (h + 1) * DH)
        wqk_h = np.concatenate([wq_s[:, sl], Wk[:, sl]], axis=1)
        in_maps.append({
            "qT": qT,
            "ebT": np.ascontiguousarray(np.exp(pos_bias[h]).T).astype(bf16),
            "wqk": np.ascontiguousarray(wqk_h).astype(bf16),
            "wv": np.ascontiguousarray(Wv[:, sl]).astype(bf16),
            "wout": np.ascontiguousarray(Wout[sl, :]).astype(bf16),
        })
    return in_maps


def run_device(in_maps, b=B, n=N, d=D, packed=False, trace=False, **kw):
    nc = build_nc(b, n, d, packed, n_cores=len(in_maps))
    return run_bass_kernel_spmd(nc, in_maps, list(range(len(in_maps))),
                                trace=trace, **kw)


def assemble(results, b=B, n=N, d=D):
    acc = np.zeros((b * n, d), dtype=np.float32)
    for r in results:
        acc += np.asarray(r["out"], dtype=np.float32)
    return acc.reshape(b, n, d)


def kernel(query, pos_bias, Wq, Wk, Wv, Wout):
    in_maps = make_in_maps(query, pos_bias, Wq, Wk, Wv, Wout)
    res = run_device(in_maps)
    return assemble(res.results)
